# revision 28
# baseline (speedup 1.0000x reference)
"""Trainium2 Bass kernel for sparse (top-k) multi-head causal attention.

Problem (hardcoded shapes, from the reference):
  B=32, S=512, D=512, H=8, DK=64, k_index=5 (any k<=8 supported)
  out = TopKCausalAttention(q, k, v; w_q..w_o, b_q..b_o)

Sharding: data-parallel over batch across 8 NeuronCores (4 batches/core).

Precision strategy: the top-k selection is discontinuous -- score noise
delta flips selected key sets on near-tie rows with l2 error
~0.9*sqrt(delta) (measured: fp32r scores -> 1.2e-2, fp32r projections ->
1.6e-2, both too close to the 2e-2 gate). Exact-enough selection needs
~22-bit scores. Instead of fp32 matmuls (4 cyc/row on the PE), the q/k
path uses an error-free fp16 hi/lo split: x = hi + lo captures 22
mantissa bits, fp16 products are exact in the PE's f32 accumulator, and
dropping only the lo*lo term leaves score noise ~5e-6 -> ~2e-3 l2.
Each fp32 matmul (4 cyc/row) becomes 3 fp16 matmuls (1 cyc/row each):
  - q/k projections: hi/lo of both w and x are split on the HOST (free),
    qh = Wh.x_h + Wh.x_l + Wl.x_h accumulated in one PSUM group.
  - QK^T: qhT/khT hi/lo are split at PSUM evacuation time (hi: Act copy
    with fp16 cast = the evacuation we already paid for; lo: one
    subtract op on DVE/Pool).

Per-core algorithm (per batch b and head-pair hp, heads in partition
halves 0:64/64:128):
  scores_psum[r-tile, :] = qhT.T @ khT (3 fp16 matmuls) + bf16
      identity-matmul adds the strictly-causal -1e32 mask on the
      diagonal tile; upper tiles skipped
  e = exp(scores)              (Act, PSUM->SBUF, f32)
  top8 = vector.max(e)         (DVE, one op per row-tile)
  tau = top8[:, k-1]; rows < k get tau := 0 (keep everything valid)
  pu16 = (e >= tau) * e -> fp16, accum_out Z = sum(kept)   (DVE; rows
      with fp16-tie extra keeps normalize by their true kept-sum, same
      as the reference's `probs >= thresh` semantics)
  rz = 1/Z (row 0: Z := 1)     (DVE)
  pn16 = pu16 * rz             (DVE tensor_scalar, all-fp16 4x mode)
  pT via PE transposes banked 4-wide into one PSUM tile, one wide
  evacuation per column-tile (DVE/Act alternating), then one wide attnT
  matmul per ci:
  attnT[d, r>=ci*128] += vh_ci_headslice.T @ pT_ci   (fp16)
  y[r, :] = sum_hp attnT_hp-slice.T @ w_o-slice (+ b_o)  -> DRAM out

Scheduling (the PE executes its stream strictly in order, so emission
order IS the schedule; PE busy time sits at the 2.4 GHz cycle floor and
everything else is gap-hunting):
  - two-level software pipeline: each head-pair's PE-heavy back half
    (transposes/attnT) is emitted during the next head-pair's DVE-heavy
    front half (scores/top-k/normalize), and batch b+1's projection
    groups are woven between those phases as fillers; the last batch's
    v-projections are held back to fill its own cooldown.
  - inputs ride one wide DMA per operand (single descriptor-gen slot),
    spread across the SP/Act/Pool hardware DGE queues; output DMAs go
    through the otherwise-idle SP queue.
  - engine placement balances measured busy time (PE ~201us of 238us):
    exp + hi-split + PSUM evacuations on Act, top-8 + threshold +
    normalize + half the pT evacuations on DVE, lo-split on GpSimd;
    in the last batch's tail (no proj fillers left, DVE saturated) the
    normalize flips to GpSimd and evacuations to Act/DVE idle slots.
"""

import math
import os

os.environ.setdefault("MYCRO_LOCAL_CACHE", "1")

from contextlib import ExitStack

import numpy as np

import concourse.bass as bass
import concourse.bacc as bacc
import concourse.mybir as mybir
import concourse.tile as tile
from concourse.bass_utils import run_bass_kernel_spmd

B, S, D, H = 32, 512, 512, 8
DK = D // H  # 64
NCORES = 8
BC = B // NCORES  # batches per core
RT = S // 128  # row tiles per sequence
FT = D // 128  # feature tiles
NEG = -1.0e32

F32 = mybir.dt.float32
F32R = mybir.dt.float32r
BF16 = mybir.dt.bfloat16
F16 = mybir.dt.float16

_last_nc = None

# config knobs (perf/precision iteration)
CFG = {
    "proj": "mixed",   # q/k projections: "f32" | "f32r" | "f16x2" | "mixed"
                       # (mixed = q 3-term fp16 hi/lo, k fp32r)
    "score": "f16x2",  # QK^T matmuls: "f32" | "f32r" | "f16x2"
    "score_terms": 2,  # f16x2 hi/lo terms: 3 = qhKh+qhKl+qlKh, 2 = qh_h*kh, 1 = qh_h*kh_h
    "v_dt": F16,       # v projection / attnT / y matmuls (smooth path)
    "p_dt": F16,       # dtype of normalized probs (transpose + pV path)
    "pipe": 1,         # head-pair A/B software-pipeline depth (0 or 1)
    "trace": False,
}


def _build_program(k_index: int, has_bias: dict):
    """Builds the per-core Bass program."""
    nc = bacc.Bacc(
        "TRN2", target_bir_lowering=False, debug=False, num_devices=NCORES
    )

    VDT = CFG["v_dt"]
    PDT = CFG["p_dt"]
    proj16 = CFG["proj"] == "f16x2"
    score_mode = CFG["score"]

    # --- DRAM I/O -------------------------------------------------------
    mixed = CFG["proj"] == "mixed"
    if proj16:
        qTh = nc.dram_tensor("qTh", (BC, D, S), F16, kind="ExternalInput").ap()
        qTl = nc.dram_tensor("qTl", (BC, D, S), F16, kind="ExternalInput").ap()
        kTh = nc.dram_tensor("kTh", (BC, D, S), F16, kind="ExternalInput").ap()
        kTl = nc.dram_tensor("kTl", (BC, D, S), F16, kind="ExternalInput").ap()
        wqh = nc.dram_tensor("wqh", (D, D), F16, kind="ExternalInput").ap()
        wql = nc.dram_tensor("wql", (D, D), F16, kind="ExternalInput").ap()
        wkh = nc.dram_tensor("wkh", (D, D), F16, kind="ExternalInput").ap()
        wkl = nc.dram_tensor("wkl", (D, D), F16, kind="ExternalInput").ap()
    elif mixed:
        # Q path: error-free fp16 hi/lo 3-term projection (score noise ~0,
        # and 2-term scores never read qh's lo half so its evac is 1 op).
        # K path: fp32r projection (1-cyc/row); its ~11-bit operand rounding
        # is half the noise of rounding both sides.
        qTh = nc.dram_tensor("qTh", (BC, D, S), F16, kind="ExternalInput").ap()
        qTl = nc.dram_tensor("qTl", (BC, D, S), F16, kind="ExternalInput").ap()
        wqh = nc.dram_tensor("wqh", (D, D), F16, kind="ExternalInput").ap()
        wql = nc.dram_tensor("wql", (D, D), F16, kind="ExternalInput").ap()
        kT = nc.dram_tensor("kT", (BC, D, S), F32R, kind="ExternalInput").ap()
        wk = nc.dram_tensor("wk", (D, D), F32R, kind="ExternalInput").ap()
    else:
        PJDT = F32R if CFG["proj"] == "f32r" else F32
        qT = nc.dram_tensor("qT", (BC, D, S), PJDT, kind="ExternalInput").ap()
        kT = nc.dram_tensor("kT", (BC, D, S), PJDT, kind="ExternalInput").ap()
        wq = nc.dram_tensor("wq", (D, D), PJDT, kind="ExternalInput").ap()
        wk = nc.dram_tensor("wk", (D, D), PJDT, kind="ExternalInput").ap()
    vT = nc.dram_tensor("vT", (BC, D, S), VDT, kind="ExternalInput").ap()
    wv = nc.dram_tensor("wv", (D, D), VDT, kind="ExternalInput").ap()
    wo = nc.dram_tensor("wo", (D, D), VDT, kind="ExternalInput").ap()
    bias_aps = {}
    for name in ("bq", "bk", "bv", "bo"):
        if has_bias[name]:
            bias_aps[name] = nc.dram_tensor(
                name, (1, D), F32, kind="ExternalInput"
            ).ap()
    out = nc.dram_tensor("out", (BC, S, D), F32, kind="ExternalOutput").ap()

    # --- inline constants ----------------------------------------------
    ident_np = np.eye(128, dtype=np.float32)
    # additive strict-causal mask for a diagonal tile: M[r, c] = NEG if c >= r
    mask_np = np.where(
        np.arange(128)[None, :] >= np.arange(128)[:, None], NEG, 0.0
    ).astype(np.float32)
    ident_p = nc.inline_tensor(
        ident_np.astype(mybir.dt.np(PDT)), name="identp"
    ).ap()
    ident_b = nc.inline_tensor(
        ident_np.astype(mybir.dt.np(BF16)), name="identb"
    ).ap()
    maskT_b = nc.inline_tensor(
        mask_np.T.copy().astype(mybir.dt.np(BF16)), name="maskT"
    ).ap()
    ones_row = nc.inline_tensor(
        np.ones((1, S), dtype=np.float32), name="onesrow"
    ).ap()

    with tile.TileContext(nc) as tc, ExitStack() as ctx:
        # ---------------- pools ----------------
        consts = ctx.enter_context(tc.tile_pool(name="consts", bufs=1))
        xpool = ctx.enter_context(tc.tile_pool(name="xpool", bufs=2))
        projpool = ctx.enter_context(tc.tile_pool(name="projpool", bufs=2))
        epool = ctx.enter_context(tc.tile_pool(name="epool", bufs=20))
        ppool = ctx.enter_context(tc.tile_pool(name="ppool", bufs=10))
        pnpool = ctx.enter_context(tc.tile_pool(name="pnpool", bufs=12))
        ptpool = ctx.enter_context(tc.tile_pool(name="ptpool", bufs=12))
        smallpool = ctx.enter_context(tc.tile_pool(name="smallpool", bufs=4))
        atpool = ctx.enter_context(tc.tile_pool(name="atpool", bufs=3))
        ypool = ctx.enter_context(tc.tile_pool(name="ypool", bufs=3))

        # 8 PSUM banks: proj 2 + scores 2 + transpose 2 + attnT 1 + y 1.
        # attnT tolerates a single buffer (its evacuation finishes ~8us
        # before the next head-pair's accumulation starts); the transpose
        # pool wants 2 so PE transposes don't serialize behind evacuations.
        ps_proj = ctx.enter_context(tc.tile_pool(name="ps_proj", bufs=2, space="PSUM"))
        ps_sc = ctx.enter_context(tc.tile_pool(name="ps_sc", bufs=2, space="PSUM"))
        ps_pt = ctx.enter_context(tc.tile_pool(name="ps_pt", bufs=2, space="PSUM"))
        ps_at = ctx.enter_context(tc.tile_pool(name="ps_at", bufs=1, space="PSUM"))
        ps_y = ctx.enter_context(tc.tile_pool(name="ps_y", bufs=1, space="PSUM"))

        # ---------------- resident constants ----------------
        # q/k weights first, then batch 0's activations, then the rest of
        # the weights: on the DMA queue this lets the first projection
        # matmuls start earlier instead of waiting for all weight tiles.
        ET = mybir.EngineType

        def wide(ap2d):
            """[FT*128, N] dram AP -> [128, FT, N] single-DMA view."""
            return ap2d.rearrange("(f p) s -> p f s", p=128)

        def flat(t3d):
            return t3d.rearrange("p f s -> p (f s)")

        def load_x(dram, b, nm, eng, dt_=None):
            """One wide DMA for all FT column-tiles of dram[b]; returns the
            per-ft slice list."""
            t = flat(xpool.tile_from(wide(dram[b]), name=nm,
                                     forced_dma_engine=eng))
            return [t[:, f * S:(f + 1) * S] for f in range(FT)]

        def perf_w(dram2d, nm, engs):
            """Per-f-tile weight DMAs (first matmul waits on 1/4 of the load)."""
            return [consts.tile_from(
                dram2d[f * 128:(f + 1) * 128, :], name=f"{nm}{f}",
                forced_dma_engine=engs[f % len(engs)]) for f in range(FT)]

        def perf_x(dram, b, nm, engs):
            """Per-f-tile activation DMAs, tags shared across batches."""
            return [xpool.tile_from(
                dram[b, f * 128:(f + 1) * 128, :], name=f"{nm}{f}",
                forced_dma_engine=engs[f % len(engs)]) for f in range(FT)]

        if proj16:
            # weights ride single wide DMAs (1 descriptor-gen slot each,
            # not 4) so batch-0's x tiles get the startup queue slots
            def wide_w(dram2d, nm, eng):
                t = flat(consts.tile_from(wide(dram2d), name=nm,
                                          forced_dma_engine=eng))
                return [t[:, f * S:(f + 1) * S] for f in range(FT)]
            def half_w(dram2d, nm, eng):
                h0 = flat(consts.tile_from(
                    wide(dram2d[0:2 * 128, :]), name=nm + "a",
                    forced_dma_engine=eng))
                h1 = flat(consts.tile_from(
                    wide(dram2d[2 * 128:, :]), name=nm + "b",
                    forced_dma_engine=eng))
                return ([h0[:, f * S:(f + 1) * S] for f in range(2)]
                        + [h1[:, f * S:(f + 1) * S] for f in range(2)])
            _wqh = half_w(wqh, "wqh", ET.SP)
            _xqh0 = load_x(qTh, 0, "xqh", ET.Activation)
            _wkh = wide_w(wkh, "wkh", ET.Pool)
            _xkh0 = load_x(kTh, 0, "xkh", ET.SP)
            _wql = wide_w(wql, "wql", ET.Pool)
            _xql0 = load_x(qTl, 0, "xql", ET.Activation)
            _wkl = wide_w(wkl, "wkl", ET.SP)
            _xkl0 = load_x(kTl, 0, "xkl", ET.Pool)
            wq_sb = list(zip(_wqh, _wql))
            wk_sb = list(zip(_wkh, _wkl))
            preloaded = {}
            preloaded[0] = (
                list(zip(_xqh0, _xql0)),
                list(zip(_xkh0, _xkl0)),
                load_x(vT, 0, "xv", ET.Activation),
            )
        elif mixed:
            # issue order: (wqh_f, xqh_f, wk_f, xk_f) per f — the operands
            # of the f-th hi-term and k matmuls — then the q lo halves
            wqh_sb, _xqh0, wk_sb, _xk0 = [], [], [], []
            qk_engs = (ET.SP, ET.Activation, ET.Pool)
            for f in range(FT):
                wqh_sb.append(consts.tile_from(
                    wqh[f * 128:(f + 1) * 128, :], name=f"wqh{f}",
                    forced_dma_engine=qk_engs[(4 * f) % 3]))
                _xqh0.append(xpool.tile_from(
                    qTh[0, f * 128:(f + 1) * 128, :], name=f"xqh{f}",
                    forced_dma_engine=qk_engs[(4 * f + 1) % 3]))
                wk_sb.append(consts.tile_from(
                    wk[f * 128:(f + 1) * 128, :], name=f"wk{f}",
                    forced_dma_engine=qk_engs[(4 * f + 2) % 3]))
                _xk0.append(xpool.tile_from(
                    kT[0, f * 128:(f + 1) * 128, :], name=f"xk{f}",
                    forced_dma_engine=qk_engs[(4 * f + 3) % 3]))
            wql_sb = perf_w(wql, "wql", (ET.Pool, ET.SP))
            _xql0 = perf_x(qTl, 0, "xql", (ET.Activation, ET.Pool))
            wq_sb = list(zip(wqh_sb, wql_sb))
            preloaded = {}
            preloaded[0] = (
                list(zip(_xqh0, _xql0)),
                _xk0,
                load_x(vT, 0, "xv", ET.Activation),
            )
        else:
            # issue order: (wq_f, xq_f, wk_f, xk_f) per f so the f-th
            # projection matmuls unblock as early as possible
            wq_sb, _xq0, wk_sb, _xk0 = [], [], [], []
            qk_engs = (ET.SP, ET.Activation, ET.Pool)
            for f in range(FT):
                e0 = qk_engs[(4 * f) % 3]
                e1 = qk_engs[(4 * f + 1) % 3]
                e2 = qk_engs[(4 * f + 2) % 3]
                e3 = qk_engs[(4 * f + 3) % 3]
                wq_sb.append(consts.tile_from(
                    wq[f * 128:(f + 1) * 128, :], name=f"wq{f}",
                    forced_dma_engine=e0))
                _xq0.append(xpool.tile_from(
                    qT[0, f * 128:(f + 1) * 128, :], name=f"xq{f}",
                    forced_dma_engine=e1))
                wk_sb.append(consts.tile_from(
                    wk[f * 128:(f + 1) * 128, :], name=f"wk{f}",
                    forced_dma_engine=e2))
                _xk0.append(xpool.tile_from(
                    kT[0, f * 128:(f + 1) * 128, :], name=f"xk{f}",
                    forced_dma_engine=e3))
            preloaded = {}
            preloaded[0] = (
                _xq0,
                _xk0,
                load_x(vT, 0, "xv", ET.Activation),
            )
        wv_sb = [consts.tile_from(wv[f * 128:(f + 1) * 128, :], name=f"wv{f}")
                 for f in range(FT)]
        wo_sb = [consts.tile_from(wo[f * 128:(f + 1) * 128, :], name=f"wo{f}")
                 for f in range(FT)]
        identp_sb = consts.tile_from(ident_p, name="identp_sb")
        identb_sb = consts.tile_from(ident_b, name="identb_sb")
        maskT_sb = consts.tile_from(maskT_b, name="maskT_sb")
        ones_sb = consts.tile_from(ones_row, name="ones_sb")
        bias_sb = {
            nm: consts.tile_from(ap, name=f"{nm}_sb") for nm, ap in bias_aps.items()
        }

        Exp = mybir.ActivationFunctionType.Exp
        AO = mybir.AluOpType

        def make_proj_thunks(b):
            """Issues batch b's input DMAs now; returns 12 thunks (8 q/k
            projection groups interleaved per dt, then 4 v-projection
            groups) plus the qhT/khT/vh lists the thunks append into.

            The thunks are woven into the PREVIOUS batch's head-pair
            emission so the in-order PE stream has projection matmuls to
            chew on wherever the head-pair dependency chains would stall it.
            """
            if b in preloaded:
                xq, xk, xv = preloaded.pop(b)
            elif proj16:
                xq = list(zip(load_x(qTh, b, "xqh", ET.SP),
                              load_x(qTl, b, "xql", ET.Pool)))
                xk = list(zip(load_x(kTh, b, "xkh", ET.Activation),
                              load_x(kTl, b, "xkl", ET.Pool)))
                xv = load_x(vT, b, "xv", ET.Activation)
            elif mixed:
                xq = list(zip(perf_x(qTh, b, "xqh", (ET.SP, ET.Activation)),
                              perf_x(qTl, b, "xql", (ET.Activation, ET.Pool))))
                xk = perf_x(kT, b, "xk", (ET.Pool, ET.SP))
                xv = load_x(vT, b, "xv", ET.Pool)
            else:
                xq = perf_x(qT, b, "xq", (ET.SP, ET.Activation))
                xk = perf_x(kT, b, "xk", (ET.Activation, ET.Pool))
                xv = load_x(vT, b, "xv", ET.Pool)
            qhT, khT, vh = [], [], []

            def qk_term_mms(ps, w_sb_, xs, bkey, dt_, term):
                """One hi/lo term's FT matmuls of a projection group."""
                nbias = bkey in bias_sb
                sl = slice(dt_ * 128, (dt_ + 1) * 128)
                nmm = 3 * FT
                for f in range(FT):
                    wh, wl = w_sb_[f]
                    xh, xl = xs[f]
                    lhs, rhs = ((wh[:, sl], xh), (wh[:, sl], xl),
                                (wl[:, sl], xh))[term]
                    i = term * FT + f + 1
                    nc.tensor.matmul(
                        ps, lhs, rhs, start=(i == 1),
                        stop=(i == nmm and not nbias))
                if term == 2 and nbias:
                    nc.tensor.matmul(
                        ps, bias_sb[bkey][0:1, sl],
                        ones_sb, start=False, stop=True)

            def qk_group(which, w_sb_, xs, bkey, outl, dt_, ps=None):
                nbias = bkey in bias_sb
                sl = slice(dt_ * 128, (dt_ + 1) * 128)
                if proj16 or (mixed and which == "q"):
                    # hi*hi terms first: they only need the hi DMAs,
                    # letting the first matmuls start earlier
                    if ps is None:
                        ps = ps_proj.tile([128, S], F32, name="psq",
                                          tag="psproj")
                        for term in range(3):
                            qk_term_mms(ps, w_sb_, xs, bkey, dt_, term)
                else:
                    ps = ps_proj.tile([128, S], F32, name="psq", tag="psproj")
                    for f in range(FT):
                        nc.tensor.matmul(
                            ps, w_sb_[f][:, sl], xs[f],
                            start=(f == 0), stop=(f == FT - 1 and not nbias))
                    if nbias:
                        nc.tensor.matmul(
                            ps, bias_sb[bkey][0:1, sl],
                            ones_sb, start=False, stop=True)
                if score_mode == "f16x2":
                    # hi/lo fp16 split straight from PSUM: hi cast on Act,
                    # lo = ps - hi on DVE (GPSIMD cannot read PSUM). The lo
                    # half is only materialized if the configured score-term
                    # count actually reads it.
                    need_lo = CFG["score_terms"] >= (3 if which == "q" else 2)
                    th = projpool.tile([128, S], F16, name=f"{which}hTh{dt_}",
                                       tag=f"{which}hTh{dt_}")
                    nc.scalar.copy(th, ps)
                    tl = None
                    if need_lo:
                        tl = projpool.tile([128, S], F16,
                                           name=f"{which}hTl{dt_}",
                                           tag=f"{which}hTl{dt_}")
                        nc.vector.tensor_tensor(tl, ps, th, AO.subtract)
                    outl.append((th, tl))
                else:
                    sdt = F32R if score_mode == "f32r" else F32
                    t = projpool.tile([128, S], sdt, name=f"{which}hT{dt_}",
                                      tag=f"{which}hT{dt_}")
                    nc.scalar.copy(t, ps)
                    outl.append(t)

            def v_group(rt):
                ps = ps_proj.tile([128, D], F32, name="psv", tag="psproj")
                nbias = "bv" in bias_sb
                for f in range(FT):
                    nc.tensor.matmul(
                        ps, xv[f][:, rt * 128:(rt + 1) * 128], wv_sb[f],
                        start=(f == 0), stop=(f == FT - 1 and not nbias))
                if nbias:
                    nc.tensor.matmul(
                        ps, ones_sb[0:1, 0:128], bias_sb["bv"],
                        start=False, stop=True)
                t = projpool.tile([128, D], VDT, name=f"vh{rt}", tag=f"vh{rt}")
                nc.scalar.copy(t, ps)
                vh.append(t)

            def qk_pair(dt_):
                """q+k projection groups for one dt, term-interleaved across
                the two projection banks: while the q group stalls on its
                late-arriving lo tiles, the k group's hi matmuls can run.
                Used for batch 0 where operands stream in from DRAM."""
                assert proj16
                ps_q = ps_proj.tile([128, S], F32, name="psq", tag="psproj")
                ps_k = ps_proj.tile([128, S], F32, name="psq", tag="psproj")
                for term in range(3):
                    qk_term_mms(ps_q, wq_sb, xq, "bq", dt_, term)
                    qk_term_mms(ps_k, wk_sb, xk, "bk", dt_, term)
                qk_group("q", wq_sb, xq, "bq", qhT, dt_, ps=ps_q)
                qk_group("k", wk_sb, xk, "bk", khT, dt_, ps=ps_k)

            import functools
            qk_thunks = []
            for dt_ in range(FT):
                qk_thunks.append(functools.partial(
                    qk_group, "q", wq_sb, xq, "bq", qhT, dt_))
                qk_thunks.append(functools.partial(
                    qk_group, "k", wk_sb, xk, "bk", khT, dt_))
            pair_thunks = [functools.partial(qk_pair, dt_) for dt_ in range(FT)]
            v_thunks = [functools.partial(v_group, rt) for rt in range(RT)]
            return qk_thunks, v_thunks, qhT, khT, vh, pair_thunks

        def emit_scores_mm(sps, qh, kh, hp, po, ri, w):
            """QK^T matmuls for one (head, row-tile) into sps[:, 0:w]."""
            rsl = slice(ri * 128, (ri + 1) * 128)
            if score_mode == "f16x2":
                qh_h, qh_l = qh
                kh_h, kh_l = kh
                terms = ((qh_h, kh_h), (qh_h, kh_l), (qh_l, kh_h))
                for i in range(CFG["score_terms"]):
                    lq, lk = terms[i]
                    nc.tensor.matmul(
                        sps[:, 0:w], lq[po:po + 64, rsl], lk[po:po + 64, 0:w],
                        start=(i == 0), stop=False)
            else:
                # f32r at full clock runs 4 cyc/row below 256 cols: widen the
                # ri=0 matmul to 256 (junk cols 128:256 are never read)
                wm = 256 if (score_mode == "f32r" and w < 256) else w
                nc.tensor.matmul(
                    sps[:, 0:wm], qh[po:po + 64, rsl], kh[po:po + 64, 0:wm],
                    start=True, stop=False)

        def emit_A(hp, qhT, khT, last=False):
            """Front half of a head pair: scores / exp / top-8 / threshold /
            normalize. PE-light, DVE-heavy; returns the pn tiles for emit_B.

            The two heads occupy partition halves 0:64 / 64:128 of qhT/khT, so
            their K=64 QK matmuls land in different PE row groups; issuing
            them back-to-back lets them run concurrently.
            """
            pnss = [[None] * RT, [None] * RT]
            top8s = []
            zks = []
            rzs = []
            for hh in range(2):
                top8s.append(smallpool.tile(
                    [128, RT * 8], F32, name=f"top8{hh}", tag=f"top8{hh}"))
                zks.append(smallpool.tile([128, RT], F32, name=f"zk{hh}",
                                          tag=f"zk{hh}"))
                rzs.append(smallpool.tile([128, RT], F32, name=f"rz{hh}",
                                          tag=f"rz{hh}"))
            for ri in range(RT):
                w = (ri + 1) * 128
                spss = []
                # both heads' K=64 QK matmuls first (disjoint PE row groups ->
                # array-level concurrency), then the full-K mask matmuls which
                # would otherwise serialize them
                for hh in range(2):
                    sps = ps_sc.tile([128, S], F32, name="sps", tag="sps")
                    emit_scores_mm(sps, qhT[hp], khT[hp], hp, hh * 64, ri, w)
                    spss.append(sps)
                for hh in range(2):
                    nc.tensor.matmul(
                        spss[hh][:, ri * 128:(ri + 1) * 128],
                        maskT_sb, identb_sb, start=False, stop=True)
                # full per-row-tile chain: exp -> top8 -> Z -> 1/Z -> pu -> pn
                # (per-ri so each tile's normalize pipeline starts as soon as
                # its own max lands, not after all four row-tiles)
                for hh in range(2):
                    top8 = top8s[hh]
                    zk = zks[hh]
                    rz = rzs[hh]
                    e = epool.tile([128, S], F32, name="e", tag="e")
                    nc.scalar.activation(e[:, 0:w], spss[hh][:, 0:w], Exp)
                    nc.vector.max(
                        out=top8[:, ri * 8:(ri + 1) * 8], in_=e[:, 0:w])
                    if ri == 0:
                        # rows < k: tau := 0 keeps every valid entry (their Z
                        # then accumulates the full valid-row sum, matching
                        # the reference's plain-softmax rows)
                        nc.vector.memset(
                            top8[0:k_index, k_index - 1:k_index], 0.0)
                    tau = top8[:, ri * 8 + k_index - 1: ri * 8 + k_index]
                    # pu = (e >= tau) * e in fp16; accum gives the kept-sum Z
                    # (DVE-only: Pool rejects the 3-input scalar_tensor_tensor)
                    pu = ppool.tile([128, S], PDT, name="pu", tag="pu")
                    nc.vector.scalar_tensor_tensor(
                        pu[:, 0:w], e[:, 0:w], tau, e[:, 0:w],
                        op0=AO.is_ge, op1=AO.mult,
                        accum_out=zk[:, ri:ri + 1])
                    if ri == 0:
                        # row 0 is fully masked (e = 0): Z would be 0 -> 1
                        nc.vector.memset(zk[0:1, 0:1], 1.0)
                    nc.vector.reciprocal(rz[:, ri:ri + 1], zk[:, ri:ri + 1])
                    pn = pnpool.tile([128, S], PDT, name="pn", tag="pn")
                    # normalize on Pool: plain tensor_scalar is Pool-legal and
                    # Pool is otherwise idle; DVE keeps max8+threshold.
                    nc.gpsimd.tensor_scalar(
                        pn[:, 0:w], pu[:, 0:w], rz[:, ri:ri + 1], None,
                        op0=AO.mult)
                    pnss[hh][ri] = pn
            return pnss

        def emit_B(hp, pnss, vh, fill=None, last=False):
            """Back half of a head pair: p transposes / evacuations / attnT.
            PE-heavy; emitted one head-pair LATER than its emit_A so the PE
            work here overlaps the next head-pair's DVE-heavy chain.

            The per-ci transposes are banked in PAIRS (ci 0+3, ci 1+2: equal
            640-col totals) so each pair evacuates in a single wide op: same
            element count, half the per-op fixed overheads; the wide pair
            (with attnT's critical ci=0) goes to Act, the other to DVE 2x."""
            ptrows = [[None] * RT, [None] * RT]
            for hh in range(2):
                pns = pnss[hh]
                for cpair in ((0, RT - 1), (1, 2)):
                    wtot = sum((RT - ci) * 128 for ci in cpair)
                    ptb = ps_pt.tile([128, wtot], PDT, name="ptb", tag="ptb")
                    off = 0
                    for ci in cpair:
                        for ri in range(ci, RT):
                            nc.tensor.transpose(
                                ptb[:, off + (ri - ci) * 128:
                                    off + (ri - ci + 1) * 128],
                                pns[ri][:, ci * 128:(ci + 1) * 128],
                                identb_sb if PDT == BF16 else identp_sb)
                        off += (RT - ci) * 128
                    ptrow = ptpool.tile([128, wtot], PDT, name="ptrow",
                                        tag="ptrow")
                    if cpair[0] == 0:
                        nc.scalar.copy(ptrow, ptb)
                    else:
                        nc.vector.tensor_copy(ptrow, ptb)
                    off = 0
                    for ci in cpair:
                        ptrows[hh][ci] = (ptrow, off)
                        off += (RT - ci) * 128
                # projection-filler slot between the two heads' transpose
                # groups / before the attnT chain (which waits on the ci=0
                # evacuation)
                if fill is not None:
                    fill(1)
            # attnT: one wide matmul per (ci, head); the two heads' M=64
            # matmuls hit different column groups -> interleave for concurrency
            at_ps = ps_at.tile([128, S], F32, name="atps", tag="atps")
            for ci in range(RT):
                wv_ = (RT - ci) * 128
                for hh in range(2):
                    h = 2 * hp + hh
                    po = hh * 64
                    pt_t, pt_off = ptrows[hh][ci]
                    nc.tensor.matmul(
                        at_ps[po:po + 64, ci * 128:S],
                        vh[ci][:, h * DK:(h + 1) * DK],
                        pt_t[:, pt_off:pt_off + wv_],
                        start=(ci == 0), stop=(ci == RT - 1),
                        skip_group_check=True)
            at = atpool.tile([128, S], VDT, name=f"at{hp}", tag=f"at{hp}")
            if last:
                nc.vector.tensor_copy(at, at_ps)
            else:
                nc.scalar.copy(at, at_ps)
            return at

        def emit_y(b, attnT_sb):
            lastb = b == BC - 1
            for ri in range(RT):
                # in the last batch there is no next-batch projection work,
                # so the projection banks are idle: rotate y groups across
                # ps_y + ps_proj instead of serializing on one evacuation
                if lastb and ri % 2 == 1:
                    yps = ps_proj.tile([128, D], F32, name="yps", tag="psproj")
                else:
                    yps = ps_y.tile([128, D], F32, name="yps", tag="yps")
                nbias = "bo" in bias_sb
                for hp in range(FT):
                    nc.tensor.matmul(
                        yps, attnT_sb[hp][:, ri * 128:(ri + 1) * 128], wo_sb[hp],
                        start=(hp == 0), stop=(hp == FT - 1 and not nbias))
                if nbias:
                    nc.tensor.matmul(
                        yps, ones_sb[0:1, 0:128], bias_sb["bo"],
                        start=False, stop=True)
                y = ypool.tile([128, D], F32, name="y", tag="y")
                # program tail: Act still drains exp/evac backlog while DVE
                # is idle -- route the final batch's y evacuations there
                if lastb:
                    nc.vector.tensor_copy(y, yps)
                else:
                    nc.scalar.copy(y, yps)
                nc.sync.dma_start(out[b, ri * 128:(ri + 1) * 128, :], y)

        # two-level software pipeline:
        #  - head-pair level: emit_B(hp) (PE-heavy transposes/attnT) is
        #    emitted during emit_A(hp+1) (DVE-heavy scores/top-k chain), so
        #    the in-order PE stream overlaps the DVE chains everywhere --
        #    including the final batch, whose DVE work would otherwise be an
        #    exposed serial tail.
        #  - batch level: batch b+1's projection groups are woven between
        #    the A/B phases of batch b as fillers; the LAST batch's v-groups
        #    are held back to fill its own A phases.
        import collections as _c
        fq = _c.deque()

        def fill(n):
            for _ in range(n):
                if fq:
                    fq.popleft()()

        qk0, v0, qhT0, khT0, vh0, pairs0 = make_proj_thunks(0)
        for t in (pairs0 if proj16 else qk0) + v0:
            t()
        ctx = {"b": 0, "qhT": qhT0, "khT": khT0, "vh": vh0, "at": []}
        pending = None  # (ctx, hp, pnss)

        def flush_pending():
            nonlocal pending
            if pending is None:
                return
            pctx, php, ppnss = pending
            pending = None
            pctx["at"].append(emit_B(php, ppnss, pctx["vh"], fill=fill,
                                     last=pctx["b"] == BC - 1))
            if len(pctx["at"]) == FT:
                emit_y(pctx["b"], pctx["at"])

        for b in range(BC):
            last = b == BC - 1
            if not last:
                nqk, nv, nqhT, nkhT, nvh, _ = make_proj_thunks(b + 1)
                fq.extend(nqk)
                if b + 1 == BC - 1 and CFG["pipe"]:
                    # held back: the last batch's v groups fill its own A
                    # phases (legal because its B phases lag by one A)
                    held_v = nv
                else:
                    held_v = None
                    fq.extend(nv)
            elif held_v:
                fq.extend(held_v)
            for hp in range(FT):
                pnss = emit_A(hp, ctx["qhT"], ctx["khT"], last=last)
                fill(1)
                if CFG["pipe"]:
                    flush_pending()
                    pending = (ctx, hp, pnss)
                else:
                    pending = (ctx, hp, pnss)
                    flush_pending()
            if not last:
                ctx = {"b": b + 1, "qhT": nqhT, "khT": nkhT, "vh": nvh,
                       "at": []}
        flush_pending()
        while fq:
            fq.popleft()()

    nc.compile()
    return nc


def _split16(x):
    """Error-free fp16 hi/lo split: x ~= hi + lo to ~22 mantissa bits."""
    hi = x.astype(np.float16)
    lo = (x - hi.astype(np.float32)).astype(np.float16)
    return hi, lo


def kernel(**inputs):
    q = np.asarray(inputs["q"], np.float32)
    k = np.asarray(inputs["k"], np.float32)
    v = np.asarray(inputs["v"], np.float32)
    w_q = np.asarray(inputs["w_q"], np.float32)
    w_k = np.asarray(inputs["w_k"], np.float32)
    w_v = np.asarray(inputs["w_v"], np.float32)
    w_o = np.asarray(inputs["w_o"], np.float32)
    b_q = np.asarray(inputs["b_q"], np.float32)
    b_k = np.asarray(inputs["b_k"], np.float32)
    b_v = np.asarray(inputs["b_v"], np.float32)
    b_o = np.asarray(inputs["b_o"], np.float32)
    k_index = int(np.asarray(inputs["k_index"]))
    assert 1 <= k_index <= 8, f"kernel supports k_index<=8, got {k_index}"

    # fold the 1/sqrt(DK) score scaling into the q projection (exact: 2^-3)
    scale = np.float32(1.0 / math.sqrt(DK))
    w_qs = (w_q * scale).astype(np.float32)
    b_qs = (b_q * scale).astype(np.float32)

    has_bias = {
        "bq": bool(np.any(b_qs)),
        "bk": bool(np.any(b_k)),
        "bv": bool(np.any(b_v)),
        "bo": bool(np.any(b_o)),
    }

    nc = _build_program(k_index, has_bias)
    global _last_nc
    _last_nc = nc

    npv = mybir.dt.np(CFG["v_dt"])
    proj16 = CFG["proj"] == "f16x2"
    shared = {
        "wv": np.ascontiguousarray(w_v.astype(npv)),
        "wo": np.ascontiguousarray(w_o.astype(npv)),
    }
    mixed = CFG["proj"] == "mixed"
    if proj16:
        for nm, arr in (("wq", w_qs), ("wk", w_k)):
            hi, lo = _split16(arr)
            shared[nm + "h"] = np.ascontiguousarray(hi)
            shared[nm + "l"] = np.ascontiguousarray(lo)
    elif mixed:
        hi, lo = _split16(w_qs)
        shared["wqh"] = np.ascontiguousarray(hi)
        shared["wql"] = np.ascontiguousarray(lo)
        shared["wk"] = np.ascontiguousarray(w_k.astype(np.float32))
    else:
        shared["wq"] = np.ascontiguousarray(w_qs.astype(np.float32))
        shared["wk"] = np.ascontiguousarray(w_k.astype(np.float32))
    for nm, arr in (("bq", b_qs), ("bk", b_k), ("bv", b_v), ("bo", b_o)):
        if has_bias[nm]:
            shared[nm] = np.ascontiguousarray(arr.reshape(1, D).astype(np.float32))

    in_maps = []
    for c in range(NCORES):
        sl = slice(c * BC, (c + 1) * BC)
        m = dict(
            shared,
            vT=np.ascontiguousarray(v[sl].transpose(0, 2, 1).astype(npv)),
        )
        if proj16:
            for nm, arr in (("qT", q), ("kT", k)):
                hi, lo = _split16(np.ascontiguousarray(arr[sl].transpose(0, 2, 1)))
                m[nm + "h"] = np.ascontiguousarray(hi)
                m[nm + "l"] = np.ascontiguousarray(lo)
        elif mixed:
            hi, lo = _split16(np.ascontiguousarray(q[sl].transpose(0, 2, 1)))
            m["qTh"] = np.ascontiguousarray(hi)
            m["qTl"] = np.ascontiguousarray(lo)
            m["kT"] = np.ascontiguousarray(k[sl].transpose(0, 2, 1))
        else:
            m["qT"] = np.ascontiguousarray(q[sl].transpose(0, 2, 1))
            m["kT"] = np.ascontiguousarray(k[sl].transpose(0, 2, 1))
        in_maps.append(m)

    res = run_bass_kernel_spmd(
        nc, in_maps, core_ids=list(range(NCORES)), trace=CFG["trace"]
    )
    out = np.concatenate([r["out"] for r in res.results], axis=0)
    kernel.last_result = res
    return out

